# revision 1
# baseline (speedup 1.0000x reference)
import sys

sys.path.insert(0, "/opt/trn_rl_repo")

import numpy as np

import concourse.bass as bass  # noqa: F401  (bass types used via bacc/tile)
import concourse.tile as tile
from concourse import bacc, mybir
from concourse.bass_utils import run_bass_kernel_spmd

H, N = 8, 2048
IN_DIM, COND_DIM, LAYER, OUT_DIM = 3, 128, 64, 3

F16 = mybir.dt.float16
F32 = mybir.dt.float32

_NC_CACHE = {}


def _build_nc():
    if "nc" in _NC_CACHE:
        return _NC_CACHE["nc"]

    nc = bacc.Bacc("TRN2", target_bir_lowering=False, debug=False, num_devices=H)

    # ---- DRAM I/O (per-core; host pre-packs layouts) ----
    d_condT = nc.dram_tensor("condT", [COND_DIM, N], F16, kind="ExternalInput")
    d_hpD0 = nc.dram_tensor("hpD0", [128, N], F16, kind="ExternalInput")
    d_WT0 = nc.dram_tensor("WT0", [128, 2, 128], F16, kind="ExternalInput")
    d_B0 = nc.dram_tensor("B0", [128, 2], F32, kind="ExternalInput")
    d_R0 = nc.dram_tensor("R0", [128, 2, 128], F16, kind="ExternalInput")
    d_WT1 = nc.dram_tensor("WT1", [128, 32, 128], F16, kind="ExternalInput")
    d_B1 = nc.dram_tensor("B1", [128, 32], F32, kind="ExternalInput")
    d_WT2 = nc.dram_tensor("WT2", [128, 32, 128], F16, kind="ExternalInput")
    d_B2 = nc.dram_tensor("B2", [128, 32], F32, kind="ExternalInput")
    d_R2 = nc.dram_tensor("R2", [128, 32, 128], F16, kind="ExternalInput")
    d_W64d1 = nc.dram_tensor("W64d1", [128, 128], F16, kind="ExternalInput")
    d_b64d1 = nc.dram_tensor("b64d1", [128, 1], F32, kind="ExternalInput")
    d_W64d2 = nc.dram_tensor("W64d2", [128, 128], F16, kind="ExternalInput")
    d_b64d2 = nc.dram_tensor("b64d2", [128, 1], F32, kind="ExternalInput")
    d_WT3A = nc.dram_tensor("WT3A", [128, 128], F16, kind="ExternalInput")
    d_WT3B = nc.dram_tensor("WT3B", [128, 64], F16, kind="ExternalInput")
    d_W364 = nc.dram_tensor("W364", [128, 3], F16, kind="ExternalInput")
    d_B3A = nc.dram_tensor("B3A", [128, 1], F32, kind="ExternalInput")
    d_B3B = nc.dram_tensor("B3B", [64, 1], F32, kind="ExternalInput")
    d_b643 = nc.dram_tensor("b643", [3, 1], F32, kind="ExternalInput")
    d_R3A = nc.dram_tensor("R3A", [128, 3], F16, kind="ExternalInput")
    d_R3B = nc.dram_tensor("R3B", [64, 3], F16, kind="ExternalInput")
    d_out = nc.dram_tensor("o", [OUT_DIM, N], F32, kind="ExternalOutput")

    with tile.TileContext(nc) as tc:
        with (
            tc.tile_pool(name="consts", bufs=1) as consts,
            tc.tile_pool(name="hpd", bufs=2) as hpd_pool,
            tc.tile_pool(name="wsb", bufs=4) as wsb_pool,
            tc.tile_pool(name="u", bufs=4) as u_pool,
            tc.tile_pool(name="osb", bufs=1) as osb_pool,
            tc.tile_pool(name="psg", bufs=2, space="PSUM") as psg,
            tc.tile_pool(name="pso", bufs=1, space="PSUM") as pso,
        ):
            def cload(dram, shape, dtype, tag):
                t = consts.tile(shape, dtype, tag=tag)
                nc.sync.dma_start(out=t, in_=dram[tuple(slice(None) for _ in shape)])
                return t

            condT = cload(d_condT, [COND_DIM, N], F16, "condT")
            hpD0 = cload(d_hpD0, [128, N], F16, "hpD0")
            WT0 = cload(d_WT0, [128, 2, 128], F16, "WT0")
            B0 = cload(d_B0, [128, 2], F32, "B0")
            R0 = cload(d_R0, [128, 2, 128], F16, "R0")
            WT1 = cload(d_WT1, [128, 32, 128], F16, "WT1")
            B1 = cload(d_B1, [128, 32], F32, "B1")
            R2 = cload(d_R2, [128, 32, 128], F16, "R2")
            W64d1 = cload(d_W64d1, [128, 128], F16, "W64d1")
            b64d1 = cload(d_b64d1, [128, 1], F32, "b64d1")
            WT2 = cload(d_WT2, [128, 32, 128], F16, "WT2")
            B2 = cload(d_B2, [128, 32], F32, "B2")
            W64d2 = cload(d_W64d2, [128, 128], F16, "W64d2")
            b64d2 = cload(d_b64d2, [128, 1], F32, "b64d2")
            WT3A = cload(d_WT3A, [128, 128], F16, "WT3A")
            WT3B = cload(d_WT3B, [128, 64], F16, "WT3B")
            W364 = cload(d_W364, [128, 3], F16, "W364")
            B3A = cload(d_B3A, [128, 1], F32, "B3A")
            B3B = cload(d_B3B, [64, 1], F32, "B3B")
            b643 = cload(d_b643, [3, 1], F32, "b643")
            R3A = cload(d_R3A, [128, 3], F16, "R3A")
            R3B = cload(d_R3B, [64, 3], F16, "R3B")

            Ident = mybir.ActivationFunctionType.Identity
            Relu = mybir.ActivationFunctionType.Relu

            def big_layer(WT, B, W64d, b64d, hp_in):
                """One 64->64 meta layer. Returns hpD (128,N) fp16 with rows
                duplicated (k and k+64 hold the same relu'd output row)."""
                po = pso.tile([128, N], F32, tag="po")
                # homogeneous column contribution, duplicated rows; opens
                # the accumulation group on every 512-col chunk
                for s in range(4):
                    nc.tensor.matmul(
                        po[:, s * 512:(s + 1) * 512],
                        W64d,
                        condT[:, s * 512:(s + 1) * 512],
                        start=True,
                        stop=False,
                    )
                for p in range(32):
                    for nb in range(2):
                        c0 = nb * 1024
                        pg = psg.tile([128, 1024], F32, tag="pg")
                        for s in range(2):
                            nc.tensor.matmul(
                                pg[:, s * 512:(s + 1) * 512],
                                WT[:, p, :],
                                condT[:, c0 + s * 512:c0 + (s + 1) * 512],
                                start=True,
                                stop=True,
                            )
                        wsb = wsb_pool.tile([128, 1024], F16, tag="wsb")
                        nc.scalar.activation(
                            wsb, pg, Ident, bias=B[:, p:p + 1], scale=1.0
                        )
                        u = u_pool.tile([128, 1024], F16, tag="u")
                        nc.vector.tensor_mul(u, wsb, hp_in[:, c0:c0 + 1024])
                        for s in range(2):
                            nc.tensor.matmul(
                                po[:, c0 + s * 512:c0 + (s + 1) * 512],
                                R2[:, p, :],
                                u[:, s * 512:(s + 1) * 512],
                                start=False,
                                stop=(p == 31),
                            )
                hp_out = hpd_pool.tile([128, N], F16, tag="hpd")
                nc.scalar.activation(hp_out, po, Relu, bias=b64d[:, 0:1], scale=1.0)
                return hp_out

            # ---- layer 0 (3 -> 64) ----
            po = pso.tile([128, N], F32, tag="po")
            for t in range(2):
                for nb in range(2):
                    c0 = nb * 1024
                    pg = psg.tile([128, 1024], F32, tag="pg")
                    for s in range(2):
                        nc.tensor.matmul(
                            pg[:, s * 512:(s + 1) * 512],
                            WT0[:, t, :],
                            condT[:, c0 + s * 512:c0 + (s + 1) * 512],
                            start=True,
                            stop=True,
                        )
                    wsb = wsb_pool.tile([128, 1024], F16, tag="wsb")
                    nc.scalar.activation(wsb, pg, Ident, bias=B0[:, t:t + 1], scale=1.0)
                    u = u_pool.tile([128, 1024], F16, tag="u")
                    nc.vector.tensor_mul(u, wsb, hpD0[:, c0:c0 + 1024])
                    for s in range(2):
                        nc.tensor.matmul(
                            po[:, c0 + s * 512:c0 + (s + 1) * 512],
                            R0[:, t, :],
                            u[:, s * 512:(s + 1) * 512],
                            start=(t == 0),
                            stop=(t == 1),
                        )
            hpD1 = hpd_pool.tile([128, N], F16, tag="hpd")
            nc.scalar.activation(hpD1, po, Relu, bias=0.0, scale=1.0)

            # ---- layers 1, 2 (64 -> 64) ----
            hpD2 = big_layer(WT1, B1, W64d1, b64d1, hpD1)
            hpD3 = big_layer(WT2, B2, W64d2, b64d2, hpD2)

            # ---- layer 3 (64 -> 3) ----
            po3 = pso.tile([128, N], F32, tag="po")
            for s in range(4):
                nc.tensor.matmul(
                    po3[0:3, s * 512:(s + 1) * 512],
                    W364,
                    condT[:, s * 512:(s + 1) * 512],
                    start=True,
                    stop=False,
                )
            # tile A: outputs o=0,1
            for nb in range(2):
                c0 = nb * 1024
                pg = psg.tile([128, 1024], F32, tag="pg")
                for s in range(2):
                    nc.tensor.matmul(
                        pg[:, s * 512:(s + 1) * 512],
                        WT3A,
                        condT[:, c0 + s * 512:c0 + (s + 1) * 512],
                        start=True,
                        stop=True,
                    )
                wsb = wsb_pool.tile([128, 1024], F16, tag="wsb")
                nc.scalar.activation(wsb, pg, Ident, bias=B3A[:, 0:1], scale=1.0)
                u = u_pool.tile([128, 1024], F16, tag="u")
                nc.vector.tensor_mul(u, wsb, hpD3[:, c0:c0 + 1024])
                for s in range(2):
                    nc.tensor.matmul(
                        po3[0:3, c0 + s * 512:c0 + (s + 1) * 512],
                        R3A,
                        u[:, s * 512:(s + 1) * 512],
                        start=False,
                        stop=False,
                    )
            # tile B: output o=2
            for nb in range(2):
                c0 = nb * 1024
                pg = psg.tile([128, 1024], F32, tag="pg")
                for s in range(2):
                    nc.tensor.matmul(
                        pg[0:64, s * 512:(s + 1) * 512],
                        WT3B,
                        condT[:, c0 + s * 512:c0 + (s + 1) * 512],
                        start=True,
                        stop=True,
                    )
                wsb = wsb_pool.tile([128, 1024], F16, tag="wsb")
                nc.scalar.activation(
                    wsb[0:64, :], pg[0:64, :], Ident, bias=B3B[:, 0:1], scale=1.0
                )
                u = u_pool.tile([128, 1024], F16, tag="u")
                nc.vector.tensor_mul(u[0:64, :], wsb[0:64, :], hpD3[0:64, c0:c0 + 1024])
                for s in range(2):
                    nc.tensor.matmul(
                        po3[0:3, c0 + s * 512:c0 + (s + 1) * 512],
                        R3B,
                        u[0:64, s * 512:(s + 1) * 512],
                        start=False,
                        stop=(nb == 1 and s == 1),
                    )
            out_sb = osb_pool.tile([OUT_DIM, N], F32, tag="osb")
            nc.scalar.activation(out_sb, po3[0:3, :], Ident, bias=b643[:, 0:1], scale=1.0)
            nc.sync.dma_start(out=d_out[:, :], in_=out_sb)

    nc.compile()
    _NC_CACHE["nc"] = nc
    return nc


def _prep_head(x, cond, W0, b0, W1, b1, W2, b2, W3, b3):
    """Build the per-core input map (host-side layout packing)."""
    f16 = np.float16
    f32 = np.float32
    S = LAYER + 1  # 65

    def big_layer_packs(W, b):
        rows_p = np.array(
            [[(2 * p + d) * S + i for d in (0, 1) for i in range(64)] for p in range(32)]
        )  # (32, 128)
        WT = np.transpose(W[rows_p, :], (2, 0, 1)).astype(f16)  # (128c, 32p, 128j)
        B = b[rows_p].T.astype(f32)  # (128j, 32p)
        rows64 = np.array([(m % 64) * S + 64 for m in range(128)])
        W64d = W[rows64, :].T.astype(f16)  # (128c, 128m)
        b64d = b[rows64].astype(f32).reshape(128, 1)
        return WT, B, W64d, b64d

    WT1, B1, W64d1, b64d1 = big_layer_packs(W1, b1)
    WT2, B2, W64d2, b64d2 = big_layer_packs(W2, b2)

    # R2[k, p, m] = 1 if m % 64 == 2p + k//64
    k = np.arange(128)[:, None, None]
    p = np.arange(32)[None, :, None]
    m = np.arange(128)[None, None, :]
    R2 = ((m % 64) == (2 * p + k // 64)).astype(f16)

    # layer 0
    rows_t = np.array(
        [[(32 * t + oo) * 4 + i for oo in range(32) for i in range(4)] for t in range(2)]
    )  # (2, 128)
    WT0 = np.transpose(W0[rows_t, :], (2, 0, 1)).astype(f16)  # (128c, 2t, 128j)
    B0 = b0[rows_t].T.astype(f32)  # (128j, 2t)
    kk = np.arange(128)[:, None, None]
    tt = np.arange(2)[None, :, None]
    R0 = ((m % 64) == (32 * tt + kk // 4)).astype(f16)
    hpD0 = np.empty((128, N), dtype=f16)
    xT = x.T  # (3, N)
    for i in range(4):
        hpD0[i::4, :] = xT[i] if i < 3 else 1.0

    # layer 3
    rowsA = np.array([o * S + i for o in (0, 1) for i in range(64)])
    rowsB = np.array([2 * S + i for i in range(64)])
    rows64_3 = np.array([o * S + 64 for o in range(3)])
    WT3A = W3[rowsA, :].T.astype(f16)
    WT3B = W3[rowsB, :].T.astype(f16)
    W364 = W3[rows64_3, :].T.astype(f16)
    B3A = b3[rowsA].astype(f32).reshape(128, 1)
    B3B = b3[rowsB].astype(f32).reshape(64, 1)
    b643 = b3[rows64_3].astype(f32).reshape(3, 1)
    kA = np.arange(128)
    R3A = (np.arange(3)[None, :] == (kA // 64)[:, None]).astype(f16)
    R3B = (np.arange(3)[None, :] == 2).astype(f16) * np.ones((64, 1), dtype=f16)

    return {
        "condT": cond.T.astype(f16).copy(),
        "hpD0": hpD0,
        "WT0": np.ascontiguousarray(WT0),
        "B0": np.ascontiguousarray(B0),
        "R0": np.ascontiguousarray(R0),
        "WT1": np.ascontiguousarray(WT1),
        "B1": np.ascontiguousarray(B1),
        "WT2": np.ascontiguousarray(WT2),
        "B2": np.ascontiguousarray(B2),
        "R2": np.ascontiguousarray(R2),
        "W64d1": np.ascontiguousarray(W64d1),
        "b64d1": b64d1,
        "W64d2": np.ascontiguousarray(W64d2),
        "b64d2": b64d2,
        "WT3A": np.ascontiguousarray(WT3A),
        "WT3B": np.ascontiguousarray(WT3B),
        "W364": np.ascontiguousarray(W364),
        "B3A": B3A,
        "B3B": B3B,
        "b643": b643,
        "R3A": np.ascontiguousarray(R3A),
        "R3B": np.ascontiguousarray(R3B),
    }


def kernel(x, cond, W0, b0, W1, b1, W2, b2, W3, b3, _trace=False):
    x = np.asarray(x, dtype=np.float32)
    cond = np.asarray(cond, dtype=np.float32)
    Ws = [np.asarray(w, dtype=np.float32) for w in (W0, W1, W2, W3)]
    bs = [np.asarray(b, dtype=np.float32) for b in (b0, b1, b2, b3)]

    nc = _build_nc()
    in_maps = [
        _prep_head(
            x[h], cond[h], Ws[0][h], bs[0][h], Ws[1][h], bs[1][h],
            Ws[2][h], bs[2][h], Ws[3][h], bs[3][h],
        )
        for h in range(H)
    ]
    res = run_bass_kernel_spmd(nc, in_maps, list(range(H)), trace=_trace)
    out = np.stack([res.results[h]["o"].T for h in range(H)]).astype(np.float32)
    if _trace:
        kernel._last_result = res
    return out


# revision 31
# speedup vs baseline: 21.4434x; 21.4434x over previous
import sys

sys.path.insert(0, "/opt/trn_rl_repo")

import contextlib

import numpy as np

import concourse.bass as bass  # noqa: F401  (bass types used via bacc/tile)
import concourse.tile as tile
from concourse import bacc, mybir
from concourse.bass_utils import run_bass_kernel_spmd

H, N = 8, 2048
IN_DIM, COND_DIM, LAYER, OUT_DIM = 3, 128, 64, 3

F16 = mybir.dt.float16
F32 = mybir.dt.float32

_NC_CACHE = {}

# Per-64-chunk mix of engines handling PSUM-evacuation + hp-multiply.
# 'split': ACT evacuates (+bias), DVE multiplies.
# 'fused': DVE scalar_tensor_tensor does (psum + bias) * hp in one pass.
MIX = {"fused": 16, "split": 48}


def _mk_schedule():
    n = sum(MIX.values())
    counts = dict(MIX)
    sched = []
    acc = {k: 0.0 for k in counts}
    for _ in range(n):
        for k in counts:
            acc[k] += counts[k] / n
        k = max(acc, key=lambda kk: acc[kk])
        acc[k] -= 1.0
        sched.append(k)
    return sched


_SCHED = _mk_schedule()


def _build_nc(reps=1, dynamic=False):
    key = (reps, dynamic)
    if key in _NC_CACHE:
        return _NC_CACHE[key]

    nc = bacc.Bacc("TRN2", target_bir_lowering=False, debug=False, num_devices=H)

    # ---- DRAM I/O (per-core; host pre-packs layouts) ----
    d_condT = nc.dram_tensor("condT", [COND_DIM, N], F16, kind="ExternalInput")
    d_z0T = nc.dram_tensor("z0T", [128, 3, N], F16, kind="ExternalInput")
    d_W0T2 = nc.dram_tensor("W0T2", [128, 4, 64], F16, kind="ExternalInput")
    d_b0mat = nc.dram_tensor("b0mat", [4, 64], F16, kind="ExternalInput")
    d_hp0T = nc.dram_tensor("hp0T", [4, N], F16, kind="ExternalInput")
    d_WT1 = nc.dram_tensor("WT1", [128, 32, 128], F16, kind="ExternalInput")
    d_B1 = nc.dram_tensor("B1", [128, 32], F32, kind="ExternalInput")
    d_WT2 = nc.dram_tensor("WT2", [128, 32, 128], F16, kind="ExternalInput")
    d_B2 = nc.dram_tensor("B2", [128, 32], F32, kind="ExternalInput")
    d_R2 = nc.dram_tensor("R2", [128, 32, 64], F16, kind="ExternalInput")
    d_W64d1 = nc.dram_tensor("W64d1", [128, 64], F16, kind="ExternalInput")
    d_b64d1 = nc.dram_tensor("b64d1", [128, 1], F32, kind="ExternalInput")
    d_W64d2 = nc.dram_tensor("W64d2", [128, 64], F16, kind="ExternalInput")
    d_b64d2 = nc.dram_tensor("b64d2", [128, 1], F32, kind="ExternalInput")
    d_WT3A = nc.dram_tensor("WT3A", [128, 128], F16, kind="ExternalInput")
    d_WT3B = nc.dram_tensor("WT3B", [128, 64], F16, kind="ExternalInput")
    d_W364 = nc.dram_tensor("W364", [128, 3], F16, kind="ExternalInput")
    d_B3A = nc.dram_tensor("B3A", [128, 1], F32, kind="ExternalInput")
    d_B3B = nc.dram_tensor("B3B", [64, 1], F32, kind="ExternalInput")
    d_b643 = nc.dram_tensor("b643", [128, 1], F32, kind="ExternalInput")
    d_R3A = nc.dram_tensor("R3A", [128, 3], F16, kind="ExternalInput")
    d_R3B = nc.dram_tensor("R3B", [64, 3], F16, kind="ExternalInput")
    d_out = nc.dram_tensor("o", [OUT_DIM, N], F32, kind="ExternalOutput")

    with tile.TileContext(nc) as tc:
        with (
            tc.tile_pool(name="consts", bufs=1) as consts,
            tc.tile_pool(name="hpd", bufs=4) as hpd_pool,
            tc.tile_pool(name="hpt", bufs=4) as hpt_pool,
            tc.tile_pool(name="wsb", bufs=8) as wsb_pool,
            tc.tile_pool(name="u", bufs=8) as u_pool,
            tc.tile_pool(name="osb", bufs=2) as osb_pool,
            tc.tile_pool(name="psg", bufs=3, space="PSUM") as psg,
            tc.tile_pool(name="pso", bufs=2, space="PSUM") as pso,
        ):
            def cload(dram, shape, dtype, tag):
                t = consts.tile(shape, dtype, tag=tag)
                nc.sync.dma_start(out=t, in_=dram[tuple(slice(None) for _ in shape)])
                return t

            # Split the large input DMAs so first-needed slices land first:
            # single-shot compute starts after ~0.5 MB instead of ~2.6 MB.
            condT = consts.tile([COND_DIM, N], F16, tag="condT")
            nc.sync.dma_start(out=condT[:, 0:1024], in_=d_condT[:, 0:1024])
            z0T = consts.tile([128, 3, N], F16, tag="z0T")
            W0T2 = cload(d_W0T2, [128, 4, 64], F16, "W0T2")
            b0mat = cload(d_b0mat, [4, 64], F16, "b0mat")
            hp0T = cload(d_hp0T, [4, N], F16, "hp0T")
            for _i in range(3):
                nc.sync.dma_start(out=z0T[:, _i, :], in_=d_z0T[:, _i, :])
            nc.sync.dma_start(out=condT[:, 1024:N], in_=d_condT[:, 1024:N])
            WT1 = consts.tile([128, 32, 128], F16, tag="WT1")
            for _h in range(2):
                nc.sync.dma_start(
                    out=WT1[:, _h * 16:(_h + 1) * 16, :],
                    in_=d_WT1[:, _h * 16:(_h + 1) * 16, :],
                )
            B1 = cload(d_B1, [128, 32], F32, "B1")
            R2 = cload(d_R2, [128, 32, 64], F16, "R2")
            W64d1 = cload(d_W64d1, [128, 64], F16, "W64d1")
            b64d1 = cload(d_b64d1, [128, 1], F32, "b64d1")
            WT2 = cload(d_WT2, [128, 32, 128], F16, "WT2")
            B2 = cload(d_B2, [128, 32], F32, "B2")
            W64d2 = cload(d_W64d2, [128, 64], F16, "W64d2")
            b64d2 = cload(d_b64d2, [128, 1], F32, "b64d2")
            WT3A = cload(d_WT3A, [128, 128], F16, "WT3A")
            WT3B = cload(d_WT3B, [128, 64], F16, "WT3B")
            W364 = cload(d_W364, [128, 3], F16, "W364")
            B3A = cload(d_B3A, [128, 1], F32, "B3A")
            B3B = cload(d_B3B, [64, 1], F32, "B3B")
            b643 = cload(d_b643, [128, 1], F32, "b643")
            R3A = cload(d_R3A, [128, 3], F16, "R3A")
            R3B = cload(d_R3B, [64, 3], F16, "R3B")

            Ident = mybir.ActivationFunctionType.Identity
            Relu = mybir.ActivationFunctionType.Relu
            chunk_no = [0]

            def emit_apply(pg, bias_ap, hp_ap, npart=128):
                """PSUM (npart,1024) + bias, times hp -> fresh fp16 u tile."""
                kind = _SCHED[chunk_no[0] % len(_SCHED)]
                chunk_no[0] += 1
                u = u_pool.tile([128, 1024], F16, tag="u")
                if kind == "fused":
                    nc.vector.scalar_tensor_tensor(
                        u[0:npart, :], pg[0:npart, :], bias_ap, hp_ap,
                        op0=mybir.AluOpType.add, op1=mybir.AluOpType.mult,
                    )
                    return u
                wsb = wsb_pool.tile([128, 1024], F16, tag="wsb")
                nc.scalar.activation(
                    wsb[0:npart, :], pg[0:npart, :], Ident, bias=bias_ap, scale=1.0
                )
                nc.vector.tensor_mul(u[0:npart, :], wsb[0:npart, :], hp_ap)
                return u

            def red2(po, lhsT, u, p0, p1, start, stop):
                """Column-tiled pair of reduce matmuls: group A (psum rows
                0:64) takes point-cols 0:512, group B (rows 64:128) takes
                512:1024. Both share the same stationary matrix and run
                concurrently in separate PE column groups."""
                nc.tensor.matmul(
                    po[0:p0, 0:512], lhsT, u[:, 0:512],
                    start=start, stop=stop, tile_position=(0, 0),
                )
                nc.tensor.matmul(
                    po[64:64 + p0, 0:512], lhsT, u[:, 512:1024],
                    start=start, stop=stop, tile_position=(0, 64),
                )

            def dup_assemble(hp_half, hp_tmp):
                """hp_tmp (128,512): rows 0:64 = res for the phase's first 512
                points, rows 64:128 = res for its last 512. Scatter into the
                (128,1024) per-phase hp tile with row duplication via DMA."""
                nc.sync.dma_start(out=hp_half[0:64, 0:512], in_=hp_tmp[0:64, :])
                nc.sync.dma_start(out=hp_half[64:128, 0:512], in_=hp_tmp[0:64, :])
                nc.sync.dma_start(out=hp_half[0:64, 512:1024], in_=hp_tmp[64:128, :])
                nc.sync.dma_start(out=hp_half[64:128, 512:1024], in_=hp_tmp[64:128, :])

            RELU_DELAY = 6

            def big_layer(WT, B, W64d, b64d, hp_in):
                """One 64->64 meta layer. Returns two per-phase hp tiles
                (128,1024) fp16, rows duplicated. The phase-0 relu+assembly
                is emitted a few chunks into phase 1 so the ACT/sync queues
                never head-of-line block on the reduce tail."""
                hp_out = []
                finalize = {}

                def fin(nb, po):
                    hp_tmp = hpt_pool.tile([128, 512], F16, tag="hpt")
                    nc.scalar.activation(
                        hp_tmp, po, Relu, bias=b64d[:, 0:1], scale=1.0
                    )
                    hp_half = hpd_pool.tile([128, 1024], F16, tag="hpd")
                    dup_assemble(hp_half, hp_tmp)
                    hp_out.append(hp_half)

                for nb in range(2):
                    c0 = nb * 1024
                    po = pso.tile([128, 512], F32, tag="po")
                    # homogeneous-column contribution opens the group
                    nc.tensor.matmul(
                        po[0:64, :], W64d, condT[:, c0:c0 + 512],
                        start=True, stop=False, tile_position=(0, 0),
                    )
                    nc.tensor.matmul(
                        po[64:128, :], W64d, condT[:, c0 + 512:c0 + 1024],
                        start=True, stop=False, tile_position=(0, 64),
                    )
                    for p in range(32):
                        pg = psg.tile([128, 1024], F32, tag="pg")
                        for s in range(2):
                            nc.tensor.matmul(
                                pg[:, s * 512:(s + 1) * 512],
                                WT[:, p, :],
                                condT[:, c0 + s * 512:c0 + (s + 1) * 512],
                                start=True,
                                stop=True,
                            )
                        u = emit_apply(pg, B[:, p:p + 1], hp_in[nb][:, 0:1024])
                        red2(po, R2[:, p, :], u, 64, 64, False, p == 31)
                        if nb == 1 and p == RELU_DELAY:
                            fin(0, finalize[0])
                    finalize[nb] = po
                fin(1, finalize[1])
                return hp_out

            if dynamic:
                loop_cm = tc.For_i(0, reps, 1)
            else:
                loop_cm = contextlib.nullcontext()
            with loop_cm:
              for _rep in range(1 if dynamic else reps):
                # ---- layer 0 (3 -> 64): pure PE via host-built z0 ----
                # out0^T[o,n] = sum_i W0[(o,i),:] . (cond^T * hp0_i)[:,n]
                #            + sum_i b0[(o,i)] * hp0^T[i,n]
                # z0T[i] = cond^T * x^T[i] for i<3; i=3 channel is condT itself.
                hpD1 = []
                for nb in range(2):
                    c0 = nb * 1024
                    po = pso.tile([128, 512], F32, tag="po")
                    for i in range(4):
                        zi = condT if i == 3 else z0T[:, i, :]
                        nc.tensor.matmul(
                            po[0:64, :], W0T2[:, i, :], zi[:, c0:c0 + 512],
                            start=(i == 0), stop=False, tile_position=(0, 0),
                        )
                        nc.tensor.matmul(
                            po[64:128, :], W0T2[:, i, :], zi[:, c0 + 512:c0 + 1024],
                            start=(i == 0), stop=False, tile_position=(0, 64),
                        )
                    nc.tensor.matmul(
                        po[0:64, :], b0mat, hp0T[:, c0:c0 + 512],
                        start=False, stop=True, tile_position=(0, 0),
                    )
                    nc.tensor.matmul(
                        po[64:128, :], b0mat, hp0T[:, c0 + 512:c0 + 1024],
                        start=False, stop=True, tile_position=(0, 64),
                    )
                    hp_tmp = hpt_pool.tile([128, 512], F16, tag="hpt")
                    nc.scalar.activation(hp_tmp, po, Relu, bias=0.0, scale=1.0)
                    hp_half = hpd_pool.tile([128, 1024], F16, tag="hpd")
                    dup_assemble(hp_half, hp_tmp)
                    hpD1.append(hp_half)

                # ---- layers 1, 2 (64 -> 64) ----
                hpD2 = big_layer(WT1, B1, W64d1, b64d1, hpD1)
                hpD3 = big_layer(WT2, B2, W64d2, b64d2, hpD2)

                # ---- layer 3 (64 -> 3) ----
                for nb in range(2):
                    c0 = nb * 1024
                    po3 = pso.tile([128, 512], F32, tag="po")
                    nc.tensor.matmul(
                        po3[0:3, :], W364, condT[:, c0:c0 + 512],
                        start=True, stop=False, tile_position=(0, 0),
                    )
                    nc.tensor.matmul(
                        po3[64:67, :], W364, condT[:, c0 + 512:c0 + 1024],
                        start=True, stop=False, tile_position=(0, 64),
                    )
                    # tile A: outputs o=0,1
                    pg = psg.tile([128, 1024], F32, tag="pg")
                    for s in range(2):
                        nc.tensor.matmul(
                            pg[:, s * 512:(s + 1) * 512],
                            WT3A,
                            condT[:, c0 + s * 512:c0 + (s + 1) * 512],
                            start=True,
                            stop=True,
                        )
                    u = emit_apply(pg, B3A[:, 0:1], hpD3[nb][:, 0:1024])
                    nc.tensor.matmul(
                        po3[0:3, :], R3A, u[:, 0:512],
                        start=False, stop=False, tile_position=(0, 0),
                    )
                    nc.tensor.matmul(
                        po3[64:67, :], R3A, u[:, 512:1024],
                        start=False, stop=False, tile_position=(0, 64),
                    )
                    # tile B: output o=2
                    pg = psg.tile([128, 1024], F32, tag="pg")
                    for s in range(2):
                        nc.tensor.matmul(
                            pg[0:64, s * 512:(s + 1) * 512],
                            WT3B,
                            condT[:, c0 + s * 512:c0 + (s + 1) * 512],
                            start=True,
                            stop=True,
                        )
                    u = emit_apply(pg, B3B[:, 0:1], hpD3[nb][0:64, 0:1024], npart=64)
                    nc.tensor.matmul(
                        po3[0:3, :], R3B, u[0:64, 0:512],
                        start=False, stop=True, tile_position=(0, 0),
                    )
                    nc.tensor.matmul(
                        po3[64:67, :], R3B, u[0:64, 512:1024],
                        start=False, stop=True, tile_position=(0, 64),
                    )
                    out_sb = osb_pool.tile([128, 512], F32, tag="osb")
                    nc.scalar.activation(
                        out_sb[0:3, :], po3[0:3, :], Ident,
                        bias=b643[0:3, 0:1], scale=1.0,
                    )
                    nc.scalar.activation(
                        out_sb[64:67, :], po3[64:67, :], Ident,
                        bias=b643[64:67, 0:1], scale=1.0,
                    )
                    nc.sync.dma_start(
                        out=d_out[:, c0:c0 + 512], in_=out_sb[0:3, :]
                    )
                    nc.sync.dma_start(
                        out=d_out[:, c0 + 512:c0 + 1024], in_=out_sb[64:67, :]
                    )

    nc.compile()
    _NC_CACHE[key] = nc
    return nc


def _prep_head(x, cond, W0, b0, W1, b1, W2, b2, W3, b3):
    """Build the per-core input map (host-side layout packing)."""
    f16 = np.float16
    f32 = np.float32
    S = LAYER + 1  # 65

    def big_layer_packs(W, b):
        rows_p = np.array(
            [[(2 * p + d) * S + i for d in (0, 1) for i in range(64)] for p in range(32)]
        )  # (32, 128)
        WT = np.transpose(W[rows_p, :], (2, 0, 1)).astype(f16)  # (128c, 32p, 128j)
        B = b[rows_p].T.astype(f32)  # (128j, 32p)
        rows64 = np.array([o * S + 64 for o in range(64)])
        W64d = W[rows64, :].T.astype(f16)  # (128c, 64o)
        b64d = b[np.array([(m % 64) * S + 64 for m in range(128)])]
        b64d = b64d.astype(f32).reshape(128, 1)
        return WT, B, W64d, b64d

    WT1, B1, W64d1, b64d1 = big_layer_packs(W1, b1)
    WT2, B2, W64d2, b64d2 = big_layer_packs(W2, b2)

    # R2[k, p, m] = 1 if m == 2p + k//64   (shared by both column groups)
    k = np.arange(128)[:, None, None]
    p = np.arange(32)[None, :, None]
    m = np.arange(64)[None, None, :]
    R2 = (m == (2 * p + k // 64)).astype(f16)

    # layer 0: host-built z0 (outer product of cond with x channels)
    xT = x.T  # (3, N)
    condTf = cond.T.astype(np.float32)  # (128, N)
    z0T = np.empty((128, 3, N), dtype=f16)
    for i in range(3):
        z0T[:, i, :] = (condTf * xT[i][None, :]).astype(f16)
    W0T2 = np.empty((128, 4, 64), dtype=f16)
    b0mat = np.empty((4, 64), dtype=f16)
    for i in range(4):
        rows = np.array([o * 4 + i for o in range(64)])
        W0T2[:, i, :] = W0[rows, :].T.astype(f16)
        b0mat[i] = b0[rows].astype(f16)
    hp0T = np.empty((4, N), dtype=f16)
    hp0T[0:3] = xT.astype(f16)
    hp0T[3] = 1.0

    # layer 3
    rowsA = np.array([o * S + i for o in (0, 1) for i in range(64)])
    rowsB = np.array([2 * S + i for i in range(64)])
    rows64_3 = np.array([o * S + 64 for o in range(3)])
    WT3A = W3[rowsA, :].T.astype(f16)
    WT3B = W3[rowsB, :].T.astype(f16)
    W364 = W3[rows64_3, :].T.astype(f16)
    B3A = b3[rowsA].astype(f32).reshape(128, 1)
    B3B = b3[rowsB].astype(f32).reshape(64, 1)
    b643 = np.zeros((128, 1), dtype=f32)
    b643[0:3, 0] = b3[rows64_3]
    b643[64:67, 0] = b3[rows64_3]
    kA = np.arange(128)
    R3A = (np.arange(3)[None, :] == (kA // 64)[:, None]).astype(f16)
    R3B = (np.arange(3)[None, :] == 2).astype(f16) * np.ones((64, 1), dtype=f16)

    return {
        "condT": cond.T.astype(f16).copy(),
        "z0T": z0T,
        "W0T2": W0T2,
        "b0mat": b0mat,
        "hp0T": hp0T,
        "WT1": np.ascontiguousarray(WT1),
        "B1": np.ascontiguousarray(B1),
        "WT2": np.ascontiguousarray(WT2),
        "B2": np.ascontiguousarray(B2),
        "R2": np.ascontiguousarray(R2),
        "W64d1": np.ascontiguousarray(W64d1),
        "b64d1": b64d1,
        "W64d2": np.ascontiguousarray(W64d2),
        "b64d2": b64d2,
        "WT3A": np.ascontiguousarray(WT3A),
        "WT3B": np.ascontiguousarray(WT3B),
        "W364": np.ascontiguousarray(W364),
        "B3A": B3A,
        "B3B": B3B,
        "b643": b643,
        "R3A": np.ascontiguousarray(R3A),
        "R3B": np.ascontiguousarray(R3B),
    }


def kernel(x, cond, W0, b0, W1, b1, W2, b2, W3, b3, _trace=False):
    x = np.asarray(x, dtype=np.float32)
    cond = np.asarray(cond, dtype=np.float32)
    Ws = [np.asarray(w, dtype=np.float32) for w in (W0, W1, W2, W3)]
    bs = [np.asarray(b, dtype=np.float32) for b in (b0, b1, b2, b3)]

    nc = _build_nc()
    in_maps = [
        _prep_head(
            x[h], cond[h], Ws[0][h], bs[0][h], Ws[1][h], bs[1][h],
            Ws[2][h], bs[2][h], Ws[3][h], bs[3][h],
        )
        for h in range(H)
    ]
    res = run_bass_kernel_spmd(nc, in_maps, list(range(H)), trace=_trace)
    out = np.stack([res.results[h]["o"].T for h in range(H)]).astype(np.float32)
    if _trace:
        kernel._last_result = res
    return out
